# revision 10
# baseline (speedup 1.0000x reference)
"""Trainium2 Bass kernel v3 for nn_EnhancedLossModule.

Per-core plan (8 cores, 256 rows each):
  - Pair scalars (r_i, r_p, <fi,fp>) via PE matmuls on transposed pair
    features + identity-mask diagonal extraction (runs first, PE warms up).
  - G = f_loc @ f_all^T via bf16 PE matmuls (PSUM fp32), double-buffered.
  - r (row norms) via squares of featT tiles + gpsimd partition_all_reduce.
  - Dense passes in bf16/fp16 SBUF (2x/4x DVE modes), labels as int16.
  - Distance rows stored fp16 to DRAM; same-label pairs packed into anchor
    slots (J=4 partners/slot) so each anchor row is gathered once per slot.
  - tensor_scalar accum trick: sum min(x, c) reduces at 4x rate; the
    -N*c correction is applied on the host / in a tiny column op.
  - Focal/label-smoothing data-parallel on the pred shard (bf16).
  - All ACT Exp/Ln ops are delayed (tile_wait_until) so the activation
    table only switches sqrt->exp once.
  - Host sums the per-core [128, NCOL] accumulators and combines losses.
"""

import math

import ml_dtypes
import numpy as np

import concourse.bacc as bacc
import concourse.bass as bass
import concourse.bass_isa as bass_isa
import concourse.tile as tile
from concourse import mybir
from concourse.bass_utils import run_bass_kernel_spmd

B, C, D = 2048, 1000, 512
N_CORES = 8
R = B // N_CORES            # 256 rows per core
RT = R // 128               # 2 row tiles
KT = D // 128               # 4 contraction tiles
J = 4                       # pair partners per anchor slot

TEMPERATURE = 0.07
C_MARGIN = 0.5
T_MARGIN = 1.0
GAMMA = 2.0
ALPHA = 0.25
SMOOTHING = 0.1
W_CONTRASTIVE = 0.1
W_TRIPLET = 0.1
W_FOCAL = 0.4
W_LABEL_SMOOTH = 0.4

MASK = 4096.0               # added to d^2 of same-label entries
XPAD = 65536.0              # invalid-slot x offset
OFF = SMOOTHING / (C - 1)
import os
EXPWAIT_MS = float(os.environ.get("EXPWAIT_MS", "0.028"))
RECIPWAIT_MS = float(os.environ.get("RECIPWAIT_MS", "0.0"))

F32 = mybir.dt.float32
BF16 = mybir.dt.bfloat16
FP16 = mybir.dt.float16
I16 = mybir.dt.int16
ALU = mybir.AluOpType
AF = mybir.ActivationFunctionType

_BUILD_CACHE: dict = {}


def _build(T0: int, T1: int):
    """T0/T1 = anchor-slot tiles whose anchors live in row-tile 0/1."""
    key = (T0, T1)
    if key in _BUILD_CACHE:
        return _BUILD_CACHE[key]
    T = T0 + T1
    G = T * J                   # pair-column groups
    TS = [T0, T1]
    SECT = 128 * (1 + 2 * J)    # psum section width per pair tile

    # accumulator columns
    COL_NEG = 0                 # 2: sum min(sim, 0.5) per row tile
    COL_SELF = 2                # 2: sum min(d', margin) per row tile
    COL_PAIR = 4                # G: sum min(d'row, x) - B*x per (t,j)
    COL_POS = COL_PAIR + G      # 1: sum valid * -ln(exp(simp/T)+1e-8)
    COL_NCO = COL_POS + 1       # 1: sum valid * min(simp, 0.5)
    COL_FOC = COL_NCO + 1       # 2: focal per row tile
    COL_LS = COL_FOC + 2        # 2: label smoothing per row tile
    NCOL = COL_LS + 2

    nc = bacc.Bacc("TRN2", target_bir_lowering=False, debug=False,
                   num_devices=N_CORES)

    # ---- DRAM I/O ----
    featT = nc.dram_tensor("featT", [D, B], BF16, kind="ExternalInput")
    ftl2 = nc.dram_tensor("ftl2", [128, KT * R], BF16, kind="ExternalInput")
    featl2 = nc.dram_tensor("featl2", [128, RT * D], BF16,
                            kind="ExternalInput")
    pred2 = nc.dram_tensor("pred2", [128, RT * C], BF16,
                           kind="ExternalInput")
    lab_row = nc.dram_tensor("lab_row", [1, B], I16, kind="ExternalInput")
    lab_loc2 = nc.dram_tensor("lab_loc2", [128, RT], F32,
                              kind="ExternalInput")
    pfiT = nc.dram_tensor("pfiT", [128, KT * T * 128], BF16,
                          kind="ExternalInput")
    pfpT = nc.dram_tensor("pfpT", [128, KT * G * 128], BF16,
                          kind="ExternalInput")
    pidx = nc.dram_tensor("pidx", [128, T * 8], I16, kind="ExternalInput")
    pval = nc.dram_tensor("pval", [128, G], F32, kind="ExternalInput")
    acc_out = nc.dram_tensor("acc_out", [128, NCOL], F32,
                             kind="ExternalOutput")

    def bcast_ap(handle, n):
        a = handle.ap()
        return bass.AP(tensor=a.tensor, offset=a.offset,
                       ap=[[0, 128], [1, n]])

    with tile.TileContext(nc) as tc:
        with (
            tc.tile_pool(name="persist", bufs=1) as persist,
            tc.tile_pool(name="scratch", bufs=2) as scratch,
            tc.tile_pool(name="small", bufs=2) as small,
            tc.tile_pool(name="gpsum", bufs=2, space="PSUM") as gpsum,
            tc.tile_pool(name="dscratch", bufs=1, space="DRAM") as dscratch,
        ):
            dp_dram = [dscratch.tile([128, B], FP16, tag=f"dp{m}",
                                     name=f"dp{m}")
                       for m in range(RT)]

            # ---------------- constants ----------------
            iota_c = persist.tile([128, C], I16)
            nc.gpsimd.iota(iota_c, pattern=[[1, C]], base=0,
                           channel_multiplier=0,
                           allow_small_or_imprecise_dtypes=True)
            iota_sq = persist.tile([128, 128], F32)
            nc.gpsimd.iota(iota_sq, pattern=[[1, 128]], base=0,
                           channel_multiplier=0,
                           allow_small_or_imprecise_dtypes=True)
            pid = persist.tile([128, 1], F32)
            nc.gpsimd.iota(pid, pattern=[[0, 1]], base=0,
                           channel_multiplier=1,
                           allow_small_or_imprecise_dtypes=True)
            ident = persist.tile([128, 128], BF16)
            nc.vector.tensor_scalar(out=ident, in0=iota_sq, scalar1=pid,
                                    scalar2=None, op0=ALU.is_equal)

            # ---------------- inputs ----------------
            # featT tiles first; pair features last (needed mid-kernel)
            ft = []
            dmaq = [nc.sync, nc.scalar]
            for k in range(KT):
                t = persist.tile([128, B], BF16, tag=f"ft{k}")
                dmaq[k % 2].dma_start(
                    out=t, in_=featT.ap()[k * 128:(k + 1) * 128, :])
                ft.append(t)
            ftl = persist.tile([128, KT * R], BF16)
            nc.sync.dma_start(out=ftl, in_=ftl2.ap())
            fl2 = persist.tile([128, RT * D], BF16)
            nc.scalar.dma_start(out=fl2, in_=featl2.ap())
            lab_r = persist.tile([1, B], I16)
            nc.scalar.dma_start(out=lab_r, in_=lab_row.ap())
            lab_b = persist.tile([128, B], I16)
            nc.gpsimd.partition_broadcast(lab_b, lab_r, channels=128)
            labl = persist.tile([128, RT], F32)
            nc.scalar.dma_start(out=labl, in_=lab_loc2.ap())
            idx_sb = persist.tile([128, T * 8], I16)
            nc.scalar.dma_start(out=idx_sb, in_=pidx.ap())
            pval_t = persist.tile([128, G], F32)
            nc.scalar.dma_start(out=pval_t, in_=pval.ap())
            fiT = persist.tile([128, KT * T * 128], BF16)
            nc.sync.dma_start(out=fiT, in_=pfiT.ap())
            pr2 = persist.tile([128, RT * C], BF16)
            nc.scalar.dma_start(out=pr2, in_=pred2.ap())
            fpT = persist.tile([128, KT * G * 128], BF16)
            nc.scalar.dma_start(out=fpT, in_=pfpT.ap())
            # junk sinks (one per engine so WAR chains stay intra-engine)
            junk_v = persist.tile([128, B], FP16)
            junk_a = persist.tile([128, B], BF16)
            junk_p = persist.tile([128, C], BF16)
            acc = persist.tile([128, NCOL], F32)
            nc.vector.memset(acc, 0.0)

            # ---------------- r row norms ----------------
            sq = []
            for k in range(KT):
                s = scratch.tile([128, B], BF16, tag=f"sq{k % 2}",
                                 name=f"sq{k}")
                nc.vector.tensor_mul(s, ft[k], ft[k])
                sq.append(s)
            ones_col = persist.tile([128, 1], BF16)
            nc.gpsimd.memset(ones_col, 1.0)
            rps = gpsum.tile([128, B], F32, tag="big", name="rps")
            for nch in range(4):
                for k in range(KT):
                    nc.tensor.matmul(
                        rps[0:1, nch * 512:(nch + 1) * 512],
                        ones_col,
                        sq[k][:, nch * 512:(nch + 1) * 512],
                        start=(k == 0), stop=(k == KT - 1),
                    )
            r_row = persist.tile([1, B], BF16)
            nc.scalar.activation(out=r_row, in_=rps[0:1, :], func=AF.Copy)
            r_b = persist.tile([128, B], BF16)
            nc.gpsimd.partition_broadcast(r_b, r_row, channels=128)
            rho_b = persist.tile([128, B], BF16)
            nc.scalar.activation(out=rho_b, in_=r_b, func=AF.Sqrt)
            s_b = persist.tile([128, B], BF16)
            with tc.tile_wait_until(RECIPWAIT_MS):
                with nc.allow_low_precision(
                        reason="1/rho in bf16; sim tolerance"):
                    nc.vector.reciprocal(out=s_b, in_=rho_b)

            # local row norms: rloc[:, m] = sum(f_loc_m^2) (fp32)
            rloc = persist.tile([128, RT], F32)
            for m in range(RT):
                nc.scalar.activation(
                    out=junk_a[:, 0:D], in_=fl2[:, m * D:(m + 1) * D],
                    func=AF.Square, accum_out=rloc[:, m:m + 1])
            rho_loc = persist.tile([128, RT], F32)
            nc.scalar.activation(out=rho_loc, in_=rloc, func=AF.Sqrt)
            s_loc = persist.tile([128, RT], F32)
            nc.vector.reciprocal(out=s_loc, in_=rho_loc)
            nhs = persist.tile([128, RT], F32)
            nc.vector.tensor_scalar(out=nhs, in0=s_loc, scalar1=-0.5,
                                    scalar2=None, op0=ALU.mult)

            # ---------------- dense: G matmuls + passes ----------------
            for m in range(RT):
                gps = gpsum.tile([128, B], F32, tag="big", name=f"gps{m}")
                for nch in range(4):
                    for k in range(KT):
                        nc.tensor.matmul(
                            gps[:, nch * 512:(nch + 1) * 512],
                            ftl[:, k * R + m * 128:k * R + (m + 1) * 128],
                            ft[k][:, nch * 512:(nch + 1) * 512],
                            start=(k == 0), stop=(k == KT - 1),
                        )
                # Gfp = -2*G + r_i (fold r_i into the copy)       [ACT]
                gfp = scratch.tile([128, B], FP16, tag="gfp")
                nc.scalar.activation(out=gfp, in_=gps, func=AF.Identity,
                                     scale=-2.0, bias=rloc[:, m:m + 1])
                # P1: mwork = (lab_b == lab_i) * MASK             [DVE 4x]
                mwork = scratch.tile([128, B], BF16, tag="mwork")
                nc.vector.tensor_scalar(out=mwork, in0=lab_b,
                                        scalar1=labl[:, m:m + 1],
                                        scalar2=MASK,
                                        op0=ALU.is_equal, op1=ALU.mult)
                # P2: rbl = mwork + r_b                           [DVE 2x]
                rbl = scratch.tile([128, B], BF16, tag="rbl")
                nc.vector.tensor_add(rbl, mwork, r_b)
                # P3: d2 = gfp + rbl                              [DVE 2x]
                d2 = scratch.tile([128, B], FP16, tag="d2")
                nc.vector.tensor_add(d2, gfp, rbl)
                # P4: dpt = sqrt(d2)                              [ACT]
                dpt = scratch.tile([128, B], FP16, tag="dpt")
                nc.scalar.activation(out=dpt, in_=d2, func=AF.Sqrt)
                # P5: store row-tile of d' to DRAM                [DMA]
                (nc.scalar if m == 0 else nc.sync).dma_start(
                    out=dp_dram[m][:, :], in_=dpt)
                # P6: accumulate sum min(d', margin)              [DVE 4x]
                nc.vector.tensor_scalar(
                    out=junk_v, in0=dpt, scalar1=float(T_MARGIN),
                    scalar2=None, op0=ALU.min, op1=ALU.add,
                    accum_out=acc[:, COL_SELF + m:COL_SELF + m + 1])
                # P7a: v = (gfp - r_i) * (-s_i/2) = G*s_i        [DVE 4x]
                vsim = scratch.tile([128, B], FP16, tag="vsim")
                nc.vector.tensor_scalar(
                    out=vsim, in0=gfp, scalar1=rloc[:, m:m + 1],
                    scalar2=nhs[:, m:m + 1],
                    op0=ALU.subtract, op1=ALU.mult)
                # P7b: sim = v * (1/rho_j)                        [DVE 2x]
                sim = scratch.tile([128, B], BF16, tag="sim")
                nc.vector.tensor_mul(sim, vsim, s_b)
                # P8: accumulate sum min(sim, 0.5)                [DVE 4x]
                nc.vector.tensor_scalar(
                    out=junk_v, in0=sim, scalar1=float(C_MARGIN),
                    scalar2=None, op0=ALU.min, op1=ALU.add,
                    accum_out=acc[:, COL_NEG + m:COL_NEG + m + 1])

            # ---------------- pair scalars via PE (PSUM recycled after dense) ----------------
            pri_t = persist.tile([128, T], F32)
            pg = persist.tile([128, G], F32)
            prp = persist.tile([128, G], F32)
            for t in range(T):
                pps = gpsum.tile([128, SECT], F32, tag="big",
                                 name=f"pps{t}")
                fi_sl = [fiT[:, k * (T * 128) + t * 128:
                             k * (T * 128) + (t + 1) * 128]
                         for k in range(KT)]
                for k in range(KT):
                    nc.tensor.matmul(pps[:, 0:128],
                                     fi_sl[k], fi_sl[k],
                                     start=(k == 0), stop=(k == KT - 1))
                for j in range(J):
                    g = t * J + j
                    fp_sl = [fpT[:, k * (G * 128) + g * 128:
                                 k * (G * 128) + (g + 1) * 128]
                             for k in range(KT)]
                    o1 = 128 + j * 256
                    for k in range(KT):
                        nc.tensor.matmul(pps[:, o1:o1 + 128],
                                         fi_sl[k], fp_sl[k],
                                         start=(k == 0), stop=(k == KT - 1))
                    for k in range(KT):
                        nc.tensor.matmul(pps[:, o1 + 128:o1 + 256],
                                         fp_sl[k], fp_sl[k],
                                         start=(k == 0), stop=(k == KT - 1))
                # diagonal extraction (DVE; PSUM src, 128-wide)
                nc.vector.scalar_tensor_tensor(
                    out=junk_v[:, 0:128], in0=pps[:, 0:128],
                    scalar=1.0, in1=ident, op0=ALU.mult, op1=ALU.mult,
                    accum_out=pri_t[:, t:t + 1])
                for j in range(J):
                    g = t * J + j
                    o1 = 128 + j * 256
                    nc.vector.scalar_tensor_tensor(
                        out=junk_v[:, 0:128], in0=pps[:, o1:o1 + 128],
                        scalar=1.0, in1=ident, op0=ALU.mult, op1=ALU.mult,
                        accum_out=pg[:, g:g + 1])
                    nc.vector.scalar_tensor_tensor(
                        out=junk_v[:, 0:128], in0=pps[:, o1 + 128:o1 + 256],
                        scalar=1.0, in1=ident, op0=ALU.mult, op1=ALU.mult,
                        accum_out=prp[:, g:g + 1])

            # pri expanded to [128, G] in (t, j) order via stride-0 AP
            pa = pri_t[:, 0:T]
            pri_g = bass.AP(tensor=pa.tensor, offset=pa.offset,
                            ap=[pa.ap[0], [pa.ap[1][0], T], [0, J]])

            # ---- pair tiny column math [128, G] fp32 ----
            rs = small.tile([128, G], F32, tag="rs")
            nc.vector.tensor_add(rs, pri_g, prp)
            d2p = small.tile([128, G], F32, tag="d2p")
            nc.vector.scalar_tensor_tensor(out=d2p, in0=pg, scalar=-2.0,
                                           in1=rs, op0=ALU.mult,
                                           op1=ALU.add)
            d2rr = small.tile([128, 2 * G], F32, tag="d2rr")
            nc.vector.tensor_scalar(out=d2rr[:, 0:G], in0=d2p, scalar1=0.0,
                                    scalar2=None, op0=ALU.max)
            nc.vector.tensor_mul(d2rr[:, G:2 * G], pri_g, prp)
            dro = small.tile([128, 2 * G], F32, tag="dro")
            nc.scalar.activation(out=dro, in_=d2rr, func=AF.Sqrt)
            dpair = dro[:, 0:G]
            rro = dro[:, G:2 * G]
            xtmp = small.tile([128, G], F32, tag="xtmp")
            nc.vector.scalar_tensor_tensor(
                out=xtmp, in0=dpair, scalar=float(T_MARGIN + XPAD),
                in1=pval_t, op0=ALU.add, op1=ALU.mult)
            xcol = persist.tile([128, G], F32)
            nc.vector.tensor_scalar(out=xcol, in0=xtmp,
                                    scalar1=float(-XPAD), scalar2=None,
                                    op0=ALU.add)
            num = small.tile([128, G], F32, tag="num")
            nc.vector.tensor_sub(num, rs, d2p)
            irr = small.tile([128, G], F32, tag="irr")
            nc.vector.reciprocal(out=irr, in_=rro)
            simp = small.tile([128, G], F32, tag="simp")
            nc.vector.scalar_tensor_tensor(out=simp, in0=num, scalar=0.5,
                                           in1=irr, op0=ALU.mult,
                                           op1=ALU.mult)
            # pos: -ln(exp(simp/T)+1e-8) ~= -simp/T (exp >> 1e-8 here)
            nc.vector.scalar_tensor_tensor(
                out=junk_v[:, 0:G], in0=simp,
                scalar=float(-1.0 / TEMPERATURE), in1=pval_t,
                op0=ALU.mult, op1=ALU.mult,
                accum_out=acc[:, COL_POS:COL_POS + 1])
            nc.vector.scalar_tensor_tensor(
                out=junk_v[:, 0:G], in0=simp, scalar=0.5, in1=pval_t,
                op0=ALU.min, op1=ALU.mult,
                accum_out=acc[:, COL_NCO:COL_NCO + 1])

            # ---------------- focal + label smoothing ----------------
            se = small.tile([128, RT], F32, tag="se")
            spred = small.tile([128, RT], F32, tag="spred")
            ptgt = small.tile([128, RT], F32, tag="ptgt")
            for m in range(RT):
                prm = pr2[:, m * C:(m + 1) * C]
                nc.vector.tensor_scalar(out=junk_v[:, 0:C], in0=prm,
                                        scalar1=1.0, scalar2=None,
                                        op0=ALU.mult, op1=ALU.add,
                                        accum_out=spred[:, m:m + 1])
                tmask = scratch.tile([128, C], BF16, tag="tmask")
                nc.vector.tensor_scalar(out=tmask, in0=iota_c,
                                        scalar1=labl[:, m:m + 1],
                                        scalar2=None, op0=ALU.is_equal)
                nc.vector.scalar_tensor_tensor(
                    out=junk_v[:, 0:C], in0=prm, scalar=1.0, in1=tmask,
                    op0=ALU.mult, op1=ALU.mult,
                    accum_out=ptgt[:, m:m + 1])

            # ---- delayed exp/ln block (single table switch) ----
            with tc.tile_wait_until(EXPWAIT_MS):
                for m in range(RT):
                    prm = pr2[:, m * C:(m + 1) * C]
                    nc.scalar.activation(out=junk_a[:, 0:C], in_=prm,
                                         func=AF.Exp,
                                         accum_out=se[:, m:m + 1])
                eptgt = small.tile([128, RT], F32, tag="eptgt")
                nc.scalar.activation(out=eptgt, in_=ptgt, func=AF.Exp)
                lse = small.tile([128, RT], F32, tag="lse")
                nc.scalar.activation(out=lse, in_=se, func=AF.Ln)
                ce = small.tile([128, RT], F32, tag="ce")
                nc.vector.tensor_sub(ce, lse, ptgt)
                invse = small.tile([128, RT], F32, tag="invse")
                nc.vector.reciprocal(out=invse, in_=se)
                pt = small.tile([128, RT], F32, tag="pt")
                nc.vector.tensor_mul(pt, eptgt, invse)
                onept = small.tile([128, RT], F32, tag="onept")
                nc.vector.tensor_scalar(out=onept, in0=pt, scalar1=-1.0,
                                        scalar2=1.0, op0=ALU.mult,
                                        op1=ALU.add)
                f2 = small.tile([128, RT], F32, tag="f2")
                nc.vector.tensor_mul(f2, onept, onept)
                nc.vector.tensor_mul(
                    acc[:, COL_FOC:COL_FOC + RT], f2, ce)
                t1 = small.tile([128, RT], F32, tag="t1")
                nc.vector.tensor_scalar(out=t1, in0=spred,
                                        scalar1=float(-OFF), scalar2=None,
                                        op0=ALU.mult)
                t2 = small.tile([128, RT], F32, tag="t2")
                nc.vector.scalar_tensor_tensor(
                    out=t2, in0=ptgt,
                    scalar=float(-(1.0 - SMOOTHING - OFF)), in1=t1,
                    op0=ALU.mult, op1=ALU.add)
                nc.vector.tensor_add(
                    acc[:, COL_LS:COL_LS + RT], lse, t2)



            # ---------------- gather + pair reductions ----------------
            # sum min(grow, x) per slot; then acc[PAIR] = pracc - B*x
            pracc = persist.tile([128, G], F32)
            toff = 0
            for m in range(RT):
                Tm = TS[m]
                if Tm == 0:
                    continue
                grow = persist.tile([128, Tm, B], FP16, tag=f"grow{m}",
                                    name=f"grow{m}")
                nc.gpsimd.dma_gather(
                    out_ap=grow,
                    in_ap=dp_dram[m][:, :],
                    idxs_ap=idx_sb[:, toff * 8:(toff + Tm) * 8],
                    num_idxs=Tm * 128,
                    num_idxs_reg=Tm * 128,
                    elem_size=B,
                )
                for tl in range(Tm):
                    for j in range(J):
                        g = (toff + tl) * J + j
                        nc.vector.tensor_scalar(
                            out=junk_v, in0=grow[:, tl, :],
                            scalar1=xcol[:, g:g + 1], scalar2=None,
                            op0=ALU.min, op1=ALU.add,
                            accum_out=pracc[:, g:g + 1])
                toff += Tm
            nc.vector.scalar_tensor_tensor(
                out=acc[:, COL_PAIR:COL_PAIR + G], in0=xcol,
                scalar=float(-B), in1=pracc, op0=ALU.mult, op1=ALU.add)

            # ---------------- writeback ----------------
            nc.sync.dma_start(out=acc_out.ap(), in_=acc)

    nc.compile()
    meta = dict(T=T, G=G, NCOL=NCOL, COL_NEG=COL_NEG, COL_SELF=COL_SELF,
                COL_PAIR=COL_PAIR, COL_POS=COL_POS, COL_NCO=COL_NCO,
                COL_FOC=COL_FOC, COL_LS=COL_LS)
    _BUILD_CACHE[key] = (nc, meta)
    return nc, meta


def _host_prep(pred, target, features):
    """Build the 8 per-core input maps."""
    pred = np.asarray(pred, dtype=np.float32)
    labels = np.asarray(target).astype(np.int64)
    features = np.asarray(features, dtype=np.float32)

    feat_bf = features.astype(ml_dtypes.bfloat16)
    featT_bf = np.ascontiguousarray(feat_bf.T)             # [D, B]
    lab_i16 = labels.astype(np.int16)

    # same-label non-self pairs grouped by anchor
    order = np.argsort(labels, kind="stable")
    sorted_lab = labels[order]
    starts = np.flatnonzero(np.r_[True, sorted_lab[1:] != sorted_lab[:-1]])
    ends = np.r_[starts[1:], len(sorted_lab)]
    partners = [[] for _ in range(B)]
    k_real = 0
    for s, e in zip(starts, ends):
        if e - s < 2:
            continue
        members = order[s:e]
        for a in members:
            for p in members:
                if p != a:
                    partners[a].append(p)
                    k_real += 1

    # anchor-slot packing: each slot = (anchor, up to J partners)
    slots = [[[] for _ in range(RT)] for _ in range(N_CORES)]
    for a in range(B):
        ps = partners[a]
        if not ps:
            continue
        c, m = a // R, (a % R) // 128
        for o in range(0, len(ps), J):
            slots[c][m].append((a, ps[o:o + J]))
    T_m = [max(1, max(math.ceil(len(slots[c][m]) / 128)
                      for c in range(N_CORES))) for m in range(RT)]
    T0, T1 = T_m
    T = T0 + T1
    G = T * J

    e1T = np.zeros((D,), ml_dtypes.bfloat16)
    e1T[0] = 1.0

    in_maps = []
    for c in range(N_CORES):
        rows = slice(c * R, (c + 1) * R)
        ftl2 = featT_bf[:, rows].reshape(KT, 128, R)
        ftl2 = np.ascontiguousarray(ftl2.transpose(1, 0, 2).reshape(
            128, KT * R))
        fl = feat_bf[rows].reshape(RT, 128, D)
        fl2 = np.ascontiguousarray(fl.transpose(1, 0, 2).reshape(
            128, RT * D))
        pr = pred[rows].astype(ml_dtypes.bfloat16).reshape(RT, 128, C)
        pr2 = np.ascontiguousarray(pr.transpose(1, 0, 2).reshape(
            128, RT * C))
        lab_loc2 = np.ascontiguousarray(
            lab_i16[rows].reshape(RT, 128).T.astype(np.float32))

        fiT = np.empty((D, T * 128), ml_dtypes.bfloat16)
        fpT = np.empty((D, G * 128), ml_dtypes.bfloat16)
        fiT[:] = e1T[:, None]
        fpT[:] = e1T[:, None]
        rowidx = np.zeros((T * 128,), np.int16)
        valid = np.zeros((128, G), np.float32)
        toff = 0
        for m in range(RT):
            sl = slots[c][m]
            for si, (a, ps) in enumerate(sl):
                t = toff + si // 128
                p = si % 128
                fiT[:, t * 128 + p] = featT_bf[:, a]
                rowidx[t * 128 + p] = a % 128
                for j, pp in enumerate(ps):
                    fpT[:, (t * J + j) * 128 + p] = featT_bf[:, pp]
                    valid[p, t * J + j] = 1.0
            toff += T_m[m]
        pfiT = np.ascontiguousarray(
            fiT.reshape(KT, 128, T * 128).transpose(1, 0, 2).reshape(
                128, KT * T * 128))
        pfpT = np.ascontiguousarray(
            fpT.reshape(KT, 128, G * 128).transpose(1, 0, 2).reshape(
                128, KT * G * 128))
        idx16 = rowidx.reshape(T, 8, 16).transpose(2, 0, 1).reshape(16, -1)
        pidx = np.ascontiguousarray(np.tile(idx16, (8, 1)))

        in_maps.append({
            "featT": featT_bf,
            "ftl2": ftl2,
            "featl2": fl2,
            "pred2": pr2,
            "lab_row": np.ascontiguousarray(lab_i16[None, :]),
            "lab_loc2": lab_loc2,
            "pfiT": pfiT,
            "pfpT": pfpT,
            "pidx": pidx,
            "pval": valid,
        })
    return in_maps, T0, T1, k_real


def _combine(results, meta, k_real):
    """Host-side scalar all-reduce + final loss combination."""
    accs = np.stack([r["acc_out"] for r in results]).astype(np.float64)
    tot = accs.sum(axis=(0, 1))                 # [NCOL]

    neg_dense = -(tot[meta["COL_NEG"]] + tot[meta["COL_NEG"] + 1]
                  - C_MARGIN * B * B)
    self_trip = -(tot[meta["COL_SELF"]] + tot[meta["COL_SELF"] + 1]
                  - T_MARGIN * B * B)
    pair_trip = -tot[meta["COL_PAIR"]:meta["COL_PAIR"] + meta["G"]].sum()
    pair_pos = tot[meta["COL_POS"]]
    nco = tot[meta["COL_NCO"]]
    focal_sum = tot[meta["COL_FOC"]] + tot[meta["COL_FOC"] + 1]
    ls_sum = tot[meta["COL_LS"]] + tot[meta["COL_LS"] + 1]

    k_tot = k_real + B
    pos_self = B * (-np.log(np.exp(1.0 / TEMPERATURE) + 1e-8))
    pos_zero = (B * B - k_tot) * (-np.log1p(1e-8))
    pos_sum = pair_pos + pos_self + pos_zero
    neg_sum = neg_dense + nco + 0.5 * B

    lc = (pos_sum + neg_sum) / (B * B)
    lt = (self_trip + pair_trip) / (B + 1e-8)
    lf = ALPHA * focal_sum / B
    ls = ls_sum / B
    total = (W_CONTRASTIVE * lc + W_TRIPLET * lt
             + W_FOCAL * lf + W_LABEL_SMOOTH * ls)
    return np.array([lc, lt, lf, ls, total], dtype=np.float32)


def kernel(pred, target, features):
    in_maps, T0, T1, k_real = _host_prep(pred, target, features)
    nc, meta = _build(T0, T1)
    res = run_bass_kernel_spmd(nc, in_maps, core_ids=list(range(N_CORES)))
    return _combine(res.results, meta, k_real)


if __name__ == "__main__":
    import reference

    inputs = reference.setup_inputs()
    expected = np.asarray(reference.reference(**inputs))
    actual = kernel(**{k: np.asarray(v) for k, v in inputs.items()})
    err = np.abs(actual - expected) / np.maximum(np.abs(expected), 1e-12)
    print("expected:", expected)
    print("actual:  ", actual)
    print("rel err: ", err)


# revision 11
# speedup vs baseline: 1.0159x; 1.0159x over previous
"""Trainium2 Bass kernel v3 for nn_EnhancedLossModule.

Per-core plan (8 cores, 256 rows each):
  - Pair scalars (r_i, r_p, <fi,fp>) via PE matmuls on transposed pair
    features + identity-mask diagonal extraction (runs first, PE warms up).
  - G = f_loc @ f_all^T via bf16 PE matmuls (PSUM fp32), double-buffered.
  - r (row norms) via squares of featT tiles + gpsimd partition_all_reduce.
  - Dense passes in bf16/fp16 SBUF (2x/4x DVE modes), labels as int16.
  - Distance rows stored fp16 to DRAM; same-label pairs packed into anchor
    slots (J=4 partners/slot) so each anchor row is gathered once per slot.
  - tensor_scalar accum trick: sum min(x, c) reduces at 4x rate; the
    -N*c correction is applied on the host / in a tiny column op.
  - Focal/label-smoothing data-parallel on the pred shard (bf16).
  - All ACT Exp/Ln ops are delayed (tile_wait_until) so the activation
    table only switches sqrt->exp once.
  - Host sums the per-core [128, NCOL] accumulators and combines losses.
"""

import math

import ml_dtypes
import numpy as np

import concourse.bacc as bacc
import concourse.bass as bass
import concourse.bass_isa as bass_isa
import concourse.tile as tile
from concourse import mybir
from concourse.bass_utils import run_bass_kernel_spmd

B, C, D = 2048, 1000, 512
N_CORES = 8
R = B // N_CORES            # 256 rows per core
RT = R // 128               # 2 row tiles
KT = D // 128               # 4 contraction tiles
J = 4                       # pair partners per anchor slot

TEMPERATURE = 0.07
C_MARGIN = 0.5
T_MARGIN = 1.0
GAMMA = 2.0
ALPHA = 0.25
SMOOTHING = 0.1
W_CONTRASTIVE = 0.1
W_TRIPLET = 0.1
W_FOCAL = 0.4
W_LABEL_SMOOTH = 0.4

MASK = 4096.0               # added to d^2 of same-label entries
XPAD = 65536.0              # invalid-slot x offset
OFF = SMOOTHING / (C - 1)
import os
EXPWAIT_MS = float(os.environ.get("EXPWAIT_MS", "0.028"))

F32 = mybir.dt.float32
BF16 = mybir.dt.bfloat16
FP16 = mybir.dt.float16
I16 = mybir.dt.int16
ALU = mybir.AluOpType
AF = mybir.ActivationFunctionType

_BUILD_CACHE: dict = {}


def _build(T0: int, T1: int):
    """T0/T1 = anchor-slot tiles whose anchors live in row-tile 0/1."""
    key = (T0, T1)
    if key in _BUILD_CACHE:
        return _BUILD_CACHE[key]
    T = T0 + T1
    G = T * J                   # pair-column groups
    TS = [T0, T1]
    SECT = 128 * (1 + 2 * J)    # psum section width per pair tile

    # accumulator columns
    COL_NEG = 0                 # 2: sum min(sim, 0.5) per row tile
    COL_SELF = 2                # 2: sum min(d', margin) per row tile
    COL_PAIR = 4                # G: sum min(d'row, x) - B*x per (t,j)
    COL_POS = COL_PAIR + G      # 1: sum valid * -ln(exp(simp/T)+1e-8)
    COL_NCO = COL_POS + 1       # 1: sum valid * min(simp, 0.5)
    COL_FOC = COL_NCO + 1       # 2: focal per row tile
    COL_LS = COL_FOC + 2        # 2: label smoothing per row tile
    NCOL = COL_LS + 2

    nc = bacc.Bacc("TRN2", target_bir_lowering=False, debug=False,
                   num_devices=N_CORES)

    # ---- DRAM I/O ----
    featT = nc.dram_tensor("featT", [D, B], BF16, kind="ExternalInput")
    ftl2 = nc.dram_tensor("ftl2", [128, KT * R], BF16, kind="ExternalInput")
    featl2 = nc.dram_tensor("featl2", [128, RT * D], BF16,
                            kind="ExternalInput")
    pred2 = nc.dram_tensor("pred2", [128, RT * C], BF16,
                           kind="ExternalInput")
    lab_row = nc.dram_tensor("lab_row", [1, B], I16, kind="ExternalInput")
    lab_loc2 = nc.dram_tensor("lab_loc2", [128, RT], F32,
                              kind="ExternalInput")
    pfiT = nc.dram_tensor("pfiT", [128, KT * T * 128], BF16,
                          kind="ExternalInput")
    pfpT = nc.dram_tensor("pfpT", [128, KT * G * 128], BF16,
                          kind="ExternalInput")
    pidx = nc.dram_tensor("pidx", [128, T * 8], I16, kind="ExternalInput")
    pval = nc.dram_tensor("pval", [128, G], F32, kind="ExternalInput")
    acc_out = nc.dram_tensor("acc_out", [128, NCOL], F32,
                             kind="ExternalOutput")

    def bcast_ap(handle, n):
        a = handle.ap()
        return bass.AP(tensor=a.tensor, offset=a.offset,
                       ap=[[0, 128], [1, n]])

    with tile.TileContext(nc) as tc:
        with (
            tc.tile_pool(name="persist", bufs=1) as persist,
            tc.tile_pool(name="scratch", bufs=2) as scratch,
            tc.tile_pool(name="small", bufs=2) as small,
            tc.tile_pool(name="gpsum", bufs=2, space="PSUM") as gpsum,
            tc.tile_pool(name="dscratch", bufs=1, space="DRAM") as dscratch,
        ):
            dp_dram = [dscratch.tile([128, B], FP16, tag=f"dp{m}",
                                     name=f"dp{m}")
                       for m in range(RT)]

            # ---------------- constants ----------------
            iota_c = persist.tile([128, C], I16)
            nc.gpsimd.iota(iota_c, pattern=[[1, C]], base=0,
                           channel_multiplier=0,
                           allow_small_or_imprecise_dtypes=True)
            iota_sq = persist.tile([128, 128], F32)
            nc.gpsimd.iota(iota_sq, pattern=[[1, 128]], base=0,
                           channel_multiplier=0,
                           allow_small_or_imprecise_dtypes=True)
            pid = persist.tile([128, 1], F32)
            nc.gpsimd.iota(pid, pattern=[[0, 1]], base=0,
                           channel_multiplier=1,
                           allow_small_or_imprecise_dtypes=True)
            ident = persist.tile([128, 128], BF16)
            nc.vector.tensor_scalar(out=ident, in0=iota_sq, scalar1=pid,
                                    scalar2=None, op0=ALU.is_equal)

            # ---------------- inputs ----------------
            # featT tiles first; pair features last (needed mid-kernel)
            ft = []
            dmaq = [nc.sync, nc.scalar]
            for k in range(KT):
                t = persist.tile([128, B], BF16, tag=f"ft{k}")
                dmaq[k % 2].dma_start(
                    out=t, in_=featT.ap()[k * 128:(k + 1) * 128, :])
                ft.append(t)
            ftl = persist.tile([128, KT * R], BF16)
            nc.sync.dma_start(out=ftl, in_=ftl2.ap())
            fl2 = persist.tile([128, RT * D], BF16)
            nc.scalar.dma_start(out=fl2, in_=featl2.ap())
            lab_r = persist.tile([1, B], I16)
            nc.scalar.dma_start(out=lab_r, in_=lab_row.ap())
            lab_b = persist.tile([128, B], I16)
            nc.gpsimd.partition_broadcast(lab_b, lab_r, channels=128)
            labl = persist.tile([128, RT], F32)
            nc.scalar.dma_start(out=labl, in_=lab_loc2.ap())
            idx_sb = persist.tile([128, T * 8], I16)
            nc.scalar.dma_start(out=idx_sb, in_=pidx.ap())
            pval_t = persist.tile([128, G], F32)
            nc.scalar.dma_start(out=pval_t, in_=pval.ap())
            fiT = persist.tile([128, KT * T * 128], BF16)
            nc.sync.dma_start(out=fiT, in_=pfiT.ap())
            pr2 = persist.tile([128, RT * C], BF16)
            nc.scalar.dma_start(out=pr2, in_=pred2.ap())
            fpT = persist.tile([128, KT * G * 128], BF16)
            nc.scalar.dma_start(out=fpT, in_=pfpT.ap())
            # junk sinks (one per engine so WAR chains stay intra-engine)
            junk_v = persist.tile([128, B], FP16)
            junk_a = persist.tile([128, B], BF16)
            junk_p = persist.tile([128, C], BF16)
            acc = persist.tile([128, NCOL], F32)
            nc.vector.memset(acc, 0.0)

            # ---------------- r row norms ----------------
            sq = []
            for k in range(KT):
                s = scratch.tile([128, B], BF16, tag=f"sq{k % 2}",
                                 name=f"sq{k}")
                nc.vector.tensor_mul(s, ft[k], ft[k])
                sq.append(s)
            ones_col = persist.tile([128, 1], BF16)
            nc.gpsimd.memset(ones_col, 1.0)
            rps = gpsum.tile([128, B], F32, tag="big", name="rps")
            for nch in range(4):
                for k in range(KT):
                    nc.tensor.matmul(
                        rps[0:1, nch * 512:(nch + 1) * 512],
                        ones_col,
                        sq[k][:, nch * 512:(nch + 1) * 512],
                        start=(k == 0), stop=(k == KT - 1),
                    )
            r_row = persist.tile([1, B], BF16)
            nc.scalar.activation(out=r_row, in_=rps[0:1, :], func=AF.Copy)
            r_b = persist.tile([128, B], BF16)
            nc.gpsimd.partition_broadcast(r_b, r_row, channels=128)
            rho_b = persist.tile([128, B], BF16)
            nc.scalar.activation(out=rho_b, in_=r_b, func=AF.Sqrt)
            s_b = persist.tile([128, B], BF16)
            with nc.allow_low_precision(reason="1/rho in bf16; sim tolerance"):
                nc.vector.reciprocal(out=s_b, in_=rho_b)

            # local row norms: rloc[:, m] = sum(f_loc_m^2) (fp32)
            rloc = persist.tile([128, RT], F32)
            for m in range(RT):
                nc.scalar.activation(
                    out=junk_a[:, 0:D], in_=fl2[:, m * D:(m + 1) * D],
                    func=AF.Square, accum_out=rloc[:, m:m + 1])
            rho_loc = persist.tile([128, RT], F32)
            nc.scalar.activation(out=rho_loc, in_=rloc, func=AF.Sqrt)
            s_loc = persist.tile([128, RT], F32)
            nc.vector.reciprocal(out=s_loc, in_=rho_loc)
            nhs = persist.tile([128, RT], F32)
            nc.vector.tensor_scalar(out=nhs, in0=s_loc, scalar1=-0.5,
                                    scalar2=None, op0=ALU.mult)

            # ---------------- dense: G matmuls + passes ----------------
            for m in range(RT):
                gps = gpsum.tile([128, B], F32, tag="big", name=f"gps{m}")
                for nch in range(4):
                    for k in range(KT):
                        nc.tensor.matmul(
                            gps[:, nch * 512:(nch + 1) * 512],
                            ftl[:, k * R + m * 128:k * R + (m + 1) * 128],
                            ft[k][:, nch * 512:(nch + 1) * 512],
                            start=(k == 0), stop=(k == KT - 1),
                        )
                # Gfp = -2*G + r_i (fold r_i into the copy)       [ACT]
                gfp = scratch.tile([128, B], FP16, tag="gfp")
                nc.scalar.activation(out=gfp, in_=gps, func=AF.Identity,
                                     scale=-2.0, bias=rloc[:, m:m + 1])
                # P1: mwork = (lab_b == lab_i) * MASK             [DVE 4x]
                mwork = scratch.tile([128, B], BF16, tag="mwork")
                nc.vector.tensor_scalar(out=mwork, in0=lab_b,
                                        scalar1=labl[:, m:m + 1],
                                        scalar2=MASK,
                                        op0=ALU.is_equal, op1=ALU.mult)
                # P2: rbl = mwork + r_b                           [DVE 2x]
                rbl = scratch.tile([128, B], BF16, tag="rbl")
                nc.vector.tensor_add(rbl, mwork, r_b)
                # P3: d2 = gfp + rbl                              [DVE 2x]
                d2 = scratch.tile([128, B], FP16, tag="d2")
                nc.vector.tensor_add(d2, gfp, rbl)
                # P4: dpt = sqrt(d2)                              [ACT]
                dpt = scratch.tile([128, B], FP16, tag="dpt")
                nc.scalar.activation(out=dpt, in_=d2, func=AF.Sqrt)
                # P5: store row-tile of d' to DRAM                [DMA]
                (nc.scalar if m == 0 else nc.sync).dma_start(
                    out=dp_dram[m][:, :], in_=dpt)
                # P6: accumulate sum min(d', margin)              [DVE 4x]
                nc.vector.tensor_scalar(
                    out=junk_v, in0=dpt, scalar1=float(T_MARGIN),
                    scalar2=None, op0=ALU.min, op1=ALU.add,
                    accum_out=acc[:, COL_SELF + m:COL_SELF + m + 1])
                # P7a: v = (gfp - r_i) * (-s_i/2) = G*s_i        [DVE 4x]
                vsim = scratch.tile([128, B], FP16, tag="vsim")
                nc.vector.tensor_scalar(
                    out=vsim, in0=gfp, scalar1=rloc[:, m:m + 1],
                    scalar2=nhs[:, m:m + 1],
                    op0=ALU.subtract, op1=ALU.mult)
                # P7b: sim = v * (1/rho_j)                        [DVE 2x]
                sim = scratch.tile([128, B], BF16, tag="sim")
                nc.vector.tensor_mul(sim, vsim, s_b)
                # P8: accumulate sum min(sim, 0.5)                [DVE 4x]
                nc.vector.tensor_scalar(
                    out=junk_v, in0=sim, scalar1=float(C_MARGIN),
                    scalar2=None, op0=ALU.min, op1=ALU.add,
                    accum_out=acc[:, COL_NEG + m:COL_NEG + m + 1])

            # ---------------- pair scalars via PE (PSUM recycled after dense) ----------------
            pri_t = persist.tile([128, T], F32)
            pg = persist.tile([128, G], F32)
            prp = persist.tile([128, G], F32)
            for t in range(T):
                pps = gpsum.tile([128, SECT], F32, tag="big",
                                 name=f"pps{t}")
                fi_sl = [fiT[:, k * (T * 128) + t * 128:
                             k * (T * 128) + (t + 1) * 128]
                         for k in range(KT)]
                for k in range(KT):
                    nc.tensor.matmul(pps[:, 0:128],
                                     fi_sl[k], fi_sl[k],
                                     start=(k == 0), stop=(k == KT - 1))
                for j in range(J):
                    g = t * J + j
                    fp_sl = [fpT[:, k * (G * 128) + g * 128:
                                 k * (G * 128) + (g + 1) * 128]
                             for k in range(KT)]
                    o1 = 128 + j * 256
                    for k in range(KT):
                        nc.tensor.matmul(pps[:, o1:o1 + 128],
                                         fi_sl[k], fp_sl[k],
                                         start=(k == 0), stop=(k == KT - 1))
                    for k in range(KT):
                        nc.tensor.matmul(pps[:, o1 + 128:o1 + 256],
                                         fp_sl[k], fp_sl[k],
                                         start=(k == 0), stop=(k == KT - 1))
                # diagonal extraction (DVE; PSUM src, 128-wide)
                nc.vector.scalar_tensor_tensor(
                    out=junk_v[:, 0:128], in0=pps[:, 0:128],
                    scalar=1.0, in1=ident, op0=ALU.mult, op1=ALU.mult,
                    accum_out=pri_t[:, t:t + 1])
                for j in range(J):
                    g = t * J + j
                    o1 = 128 + j * 256
                    nc.vector.scalar_tensor_tensor(
                        out=junk_v[:, 0:128], in0=pps[:, o1:o1 + 128],
                        scalar=1.0, in1=ident, op0=ALU.mult, op1=ALU.mult,
                        accum_out=pg[:, g:g + 1])
                    nc.vector.scalar_tensor_tensor(
                        out=junk_v[:, 0:128], in0=pps[:, o1 + 128:o1 + 256],
                        scalar=1.0, in1=ident, op0=ALU.mult, op1=ALU.mult,
                        accum_out=prp[:, g:g + 1])

            # pri expanded to [128, G] in (t, j) order via stride-0 AP
            pa = pri_t[:, 0:T]
            pri_g = bass.AP(tensor=pa.tensor, offset=pa.offset,
                            ap=[pa.ap[0], [pa.ap[1][0], T], [0, J]])

            # ---- pair tiny column math [128, G] fp32 ----
            rs = small.tile([128, G], F32, tag="rs")
            nc.vector.tensor_add(rs, pri_g, prp)
            d2p = small.tile([128, G], F32, tag="d2p")
            nc.vector.scalar_tensor_tensor(out=d2p, in0=pg, scalar=-2.0,
                                           in1=rs, op0=ALU.mult,
                                           op1=ALU.add)
            d2rr = small.tile([128, 2 * G], F32, tag="d2rr")
            nc.vector.tensor_scalar(out=d2rr[:, 0:G], in0=d2p, scalar1=0.0,
                                    scalar2=None, op0=ALU.max)
            nc.vector.tensor_mul(d2rr[:, G:2 * G], pri_g, prp)
            dro = small.tile([128, 2 * G], F32, tag="dro")
            nc.scalar.activation(out=dro, in_=d2rr, func=AF.Sqrt)
            dpair = dro[:, 0:G]
            rro = dro[:, G:2 * G]
            xtmp = small.tile([128, G], F32, tag="xtmp")
            nc.vector.scalar_tensor_tensor(
                out=xtmp, in0=dpair, scalar=float(T_MARGIN + XPAD),
                in1=pval_t, op0=ALU.add, op1=ALU.mult)
            xcol = persist.tile([128, G], F32)
            nc.vector.tensor_scalar(out=xcol, in0=xtmp,
                                    scalar1=float(-XPAD), scalar2=None,
                                    op0=ALU.add)
            num = small.tile([128, G], F32, tag="num")
            nc.vector.tensor_sub(num, rs, d2p)
            irr = small.tile([128, G], F32, tag="irr")
            nc.vector.reciprocal(out=irr, in_=rro)
            simp = small.tile([128, G], F32, tag="simp")
            nc.vector.scalar_tensor_tensor(out=simp, in0=num, scalar=0.5,
                                           in1=irr, op0=ALU.mult,
                                           op1=ALU.mult)
            # pos: -ln(exp(simp/T)+1e-8) ~= -simp/T (exp >> 1e-8 here)
            nc.vector.scalar_tensor_tensor(
                out=junk_v[:, 0:G], in0=simp,
                scalar=float(-1.0 / TEMPERATURE), in1=pval_t,
                op0=ALU.mult, op1=ALU.mult,
                accum_out=acc[:, COL_POS:COL_POS + 1])
            nc.vector.scalar_tensor_tensor(
                out=junk_v[:, 0:G], in0=simp, scalar=0.5, in1=pval_t,
                op0=ALU.min, op1=ALU.mult,
                accum_out=acc[:, COL_NCO:COL_NCO + 1])

            # ---------------- focal + label smoothing ----------------
            se = small.tile([128, RT], F32, tag="se")
            spred = small.tile([128, RT], F32, tag="spred")
            ptgt = small.tile([128, RT], F32, tag="ptgt")
            for m in range(RT):
                prm = pr2[:, m * C:(m + 1) * C]
                nc.vector.tensor_scalar(out=junk_v[:, 0:C], in0=prm,
                                        scalar1=1.0, scalar2=None,
                                        op0=ALU.mult, op1=ALU.add,
                                        accum_out=spred[:, m:m + 1])
                tmask = scratch.tile([128, C], BF16, tag="tmask")
                nc.vector.tensor_scalar(out=tmask, in0=iota_c,
                                        scalar1=labl[:, m:m + 1],
                                        scalar2=None, op0=ALU.is_equal)
                nc.vector.scalar_tensor_tensor(
                    out=junk_v[:, 0:C], in0=prm, scalar=1.0, in1=tmask,
                    op0=ALU.mult, op1=ALU.mult,
                    accum_out=ptgt[:, m:m + 1])

            # ---- delayed exp/ln block (single table switch) ----
            with tc.tile_wait_until(EXPWAIT_MS):
                for m in range(RT):
                    prm = pr2[:, m * C:(m + 1) * C]
                    nc.scalar.activation(out=junk_a[:, 0:C], in_=prm,
                                         func=AF.Exp,
                                         accum_out=se[:, m:m + 1])
                eptgt = small.tile([128, RT], F32, tag="eptgt")
                nc.scalar.activation(out=eptgt, in_=ptgt, func=AF.Exp)
                lse = small.tile([128, RT], F32, tag="lse")
                nc.scalar.activation(out=lse, in_=se, func=AF.Ln)
                ce = small.tile([128, RT], F32, tag="ce")
                nc.vector.tensor_sub(ce, lse, ptgt)
                invse = small.tile([128, RT], F32, tag="invse")
                nc.vector.reciprocal(out=invse, in_=se)
                pt = small.tile([128, RT], F32, tag="pt")
                nc.vector.tensor_mul(pt, eptgt, invse)
                onept = small.tile([128, RT], F32, tag="onept")
                nc.vector.tensor_scalar(out=onept, in0=pt, scalar1=-1.0,
                                        scalar2=1.0, op0=ALU.mult,
                                        op1=ALU.add)
                f2 = small.tile([128, RT], F32, tag="f2")
                nc.vector.tensor_mul(f2, onept, onept)
                nc.vector.tensor_mul(
                    acc[:, COL_FOC:COL_FOC + RT], f2, ce)
                t1 = small.tile([128, RT], F32, tag="t1")
                nc.vector.tensor_scalar(out=t1, in0=spred,
                                        scalar1=float(-OFF), scalar2=None,
                                        op0=ALU.mult)
                t2 = small.tile([128, RT], F32, tag="t2")
                nc.vector.scalar_tensor_tensor(
                    out=t2, in0=ptgt,
                    scalar=float(-(1.0 - SMOOTHING - OFF)), in1=t1,
                    op0=ALU.mult, op1=ALU.add)
                nc.vector.tensor_add(
                    acc[:, COL_LS:COL_LS + RT], lse, t2)



            # ---------------- gather + pair reductions ----------------
            # sum min(grow, x) per slot; then acc[PAIR] = pracc - B*x
            pracc = persist.tile([128, G], F32)
            toff = 0
            for m in range(RT):
                Tm = TS[m]
                if Tm == 0:
                    continue
                grow = persist.tile([128, Tm, B], FP16, tag=f"grow{m}",
                                    name=f"grow{m}")
                nc.gpsimd.dma_gather(
                    out_ap=grow,
                    in_ap=dp_dram[m][:, :],
                    idxs_ap=idx_sb[:, toff * 8:(toff + Tm) * 8],
                    num_idxs=Tm * 128,
                    num_idxs_reg=Tm * 128,
                    elem_size=B,
                )
                for tl in range(Tm):
                    for j in range(J):
                        g = (toff + tl) * J + j
                        nc.vector.tensor_scalar(
                            out=junk_v, in0=grow[:, tl, :],
                            scalar1=xcol[:, g:g + 1], scalar2=None,
                            op0=ALU.min, op1=ALU.add,
                            accum_out=pracc[:, g:g + 1])
                toff += Tm
            nc.vector.scalar_tensor_tensor(
                out=acc[:, COL_PAIR:COL_PAIR + G], in0=xcol,
                scalar=float(-B), in1=pracc, op0=ALU.mult, op1=ALU.add)

            # ---------------- writeback ----------------
            nc.sync.dma_start(out=acc_out.ap(), in_=acc)

    nc.compile()
    meta = dict(T=T, G=G, NCOL=NCOL, COL_NEG=COL_NEG, COL_SELF=COL_SELF,
                COL_PAIR=COL_PAIR, COL_POS=COL_POS, COL_NCO=COL_NCO,
                COL_FOC=COL_FOC, COL_LS=COL_LS)
    _BUILD_CACHE[key] = (nc, meta)
    return nc, meta


def _host_prep(pred, target, features):
    """Build the 8 per-core input maps."""
    pred = np.asarray(pred, dtype=np.float32)
    labels = np.asarray(target).astype(np.int64)
    features = np.asarray(features, dtype=np.float32)

    feat_bf = features.astype(ml_dtypes.bfloat16)
    featT_bf = np.ascontiguousarray(feat_bf.T)             # [D, B]
    lab_i16 = labels.astype(np.int16)

    # same-label non-self pairs grouped by anchor
    order = np.argsort(labels, kind="stable")
    sorted_lab = labels[order]
    starts = np.flatnonzero(np.r_[True, sorted_lab[1:] != sorted_lab[:-1]])
    ends = np.r_[starts[1:], len(sorted_lab)]
    partners = [[] for _ in range(B)]
    k_real = 0
    for s, e in zip(starts, ends):
        if e - s < 2:
            continue
        members = order[s:e]
        for a in members:
            for p in members:
                if p != a:
                    partners[a].append(p)
                    k_real += 1

    # anchor-slot packing: each slot = (anchor, up to J partners)
    slots = [[[] for _ in range(RT)] for _ in range(N_CORES)]
    for a in range(B):
        ps = partners[a]
        if not ps:
            continue
        c, m = a // R, (a % R) // 128
        for o in range(0, len(ps), J):
            slots[c][m].append((a, ps[o:o + J]))
    T_m = [max(1, max(math.ceil(len(slots[c][m]) / 128)
                      for c in range(N_CORES))) for m in range(RT)]
    T0, T1 = T_m
    T = T0 + T1
    G = T * J

    e1T = np.zeros((D,), ml_dtypes.bfloat16)
    e1T[0] = 1.0

    in_maps = []
    for c in range(N_CORES):
        rows = slice(c * R, (c + 1) * R)
        ftl2 = featT_bf[:, rows].reshape(KT, 128, R)
        ftl2 = np.ascontiguousarray(ftl2.transpose(1, 0, 2).reshape(
            128, KT * R))
        fl = feat_bf[rows].reshape(RT, 128, D)
        fl2 = np.ascontiguousarray(fl.transpose(1, 0, 2).reshape(
            128, RT * D))
        pr = pred[rows].astype(ml_dtypes.bfloat16).reshape(RT, 128, C)
        pr2 = np.ascontiguousarray(pr.transpose(1, 0, 2).reshape(
            128, RT * C))
        lab_loc2 = np.ascontiguousarray(
            lab_i16[rows].reshape(RT, 128).T.astype(np.float32))

        fiT = np.empty((D, T * 128), ml_dtypes.bfloat16)
        fpT = np.empty((D, G * 128), ml_dtypes.bfloat16)
        fiT[:] = e1T[:, None]
        fpT[:] = e1T[:, None]
        rowidx = np.zeros((T * 128,), np.int16)
        valid = np.zeros((128, G), np.float32)
        toff = 0
        for m in range(RT):
            sl = slots[c][m]
            for si, (a, ps) in enumerate(sl):
                t = toff + si // 128
                p = si % 128
                fiT[:, t * 128 + p] = featT_bf[:, a]
                rowidx[t * 128 + p] = a % 128
                for j, pp in enumerate(ps):
                    fpT[:, (t * J + j) * 128 + p] = featT_bf[:, pp]
                    valid[p, t * J + j] = 1.0
            toff += T_m[m]
        pfiT = np.ascontiguousarray(
            fiT.reshape(KT, 128, T * 128).transpose(1, 0, 2).reshape(
                128, KT * T * 128))
        pfpT = np.ascontiguousarray(
            fpT.reshape(KT, 128, G * 128).transpose(1, 0, 2).reshape(
                128, KT * G * 128))
        idx16 = rowidx.reshape(T, 8, 16).transpose(2, 0, 1).reshape(16, -1)
        pidx = np.ascontiguousarray(np.tile(idx16, (8, 1)))

        in_maps.append({
            "featT": featT_bf,
            "ftl2": ftl2,
            "featl2": fl2,
            "pred2": pr2,
            "lab_row": np.ascontiguousarray(lab_i16[None, :]),
            "lab_loc2": lab_loc2,
            "pfiT": pfiT,
            "pfpT": pfpT,
            "pidx": pidx,
            "pval": valid,
        })
    return in_maps, T0, T1, k_real


def _combine(results, meta, k_real):
    """Host-side scalar all-reduce + final loss combination."""
    accs = np.stack([r["acc_out"] for r in results]).astype(np.float64)
    tot = accs.sum(axis=(0, 1))                 # [NCOL]

    neg_dense = -(tot[meta["COL_NEG"]] + tot[meta["COL_NEG"] + 1]
                  - C_MARGIN * B * B)
    self_trip = -(tot[meta["COL_SELF"]] + tot[meta["COL_SELF"] + 1]
                  - T_MARGIN * B * B)
    pair_trip = -tot[meta["COL_PAIR"]:meta["COL_PAIR"] + meta["G"]].sum()
    pair_pos = tot[meta["COL_POS"]]
    nco = tot[meta["COL_NCO"]]
    focal_sum = tot[meta["COL_FOC"]] + tot[meta["COL_FOC"] + 1]
    ls_sum = tot[meta["COL_LS"]] + tot[meta["COL_LS"] + 1]

    k_tot = k_real + B
    pos_self = B * (-np.log(np.exp(1.0 / TEMPERATURE) + 1e-8))
    pos_zero = (B * B - k_tot) * (-np.log1p(1e-8))
    pos_sum = pair_pos + pos_self + pos_zero
    neg_sum = neg_dense + nco + 0.5 * B

    lc = (pos_sum + neg_sum) / (B * B)
    lt = (self_trip + pair_trip) / (B + 1e-8)
    lf = ALPHA * focal_sum / B
    ls = ls_sum / B
    total = (W_CONTRASTIVE * lc + W_TRIPLET * lt
             + W_FOCAL * lf + W_LABEL_SMOOTH * ls)
    return np.array([lc, lt, lf, ls, total], dtype=np.float32)


def kernel(pred, target, features):
    in_maps, T0, T1, k_real = _host_prep(pred, target, features)
    nc, meta = _build(T0, T1)
    res = run_bass_kernel_spmd(nc, in_maps, core_ids=list(range(N_CORES)))
    return _combine(res.results, meta, k_real)


if __name__ == "__main__":
    import reference

    inputs = reference.setup_inputs()
    expected = np.asarray(reference.reference(**inputs))
    actual = kernel(**{k: np.asarray(v) for k, v in inputs.items()})
    err = np.abs(actual - expected) / np.maximum(np.abs(expected), 1e-12)
    print("expected:", expected)
    print("actual:  ", actual)
    print("rel err: ", err)
